# revision 1
# baseline (speedup 1.0000x reference)
"""Trainium2 Bass kernel for nn_CLFBlock (linear -> LIF scan -> linear -> T-mean -> log_softmax).

Self-contained: hardcodes shapes T=32, B=512, D=1024, C=1000 and data-parallel
sharding of the batch dim across 8 NeuronCores.

Math notes:
  h = x @ W1.T + b1                      (computed in bf16 on the PE, fp32 accum)
  LIF (tau=2, v_th=1, hard reset to 0):
     v' = 0.5*v + 0.5*h
     s  = (v' >= 1);  v = v' * (v' < 1)
  Scan state is kept pre-halved:  hh = 0.5*h + 0.5*b1,  vh = 0.5*v, so per step
     w  = vh + hh            (tensor_tensor add, 2x DVE mode)
     mh = (w < 1) * 0.5      (one fused tensor_scalar, 4x DVE mode)
     vh = w * mh             (tensor_tensor mult, 2x DVE mode)
  Spike sum accumulates on the tensor engine: msum_psum += I @ mh_t, and
  sum_t s_t = T - 2*msum.
  y = mean_t(s_t @ W2.T + b2) = (sum_t s_t) @ W2.T / T + b2
  out = log_softmax(y, axis=1)

Layout: the tensor engine contracts along the partition axis, so both matmul
operands need the contraction dim (d / e) on partitions. The host-side shard
step hands each core its x slice already transposed ([D, T*Bc]) and the
weights transposed once ([D, D] / [D, C]), so the device needs no transposes
at all: inputs are cast fp32->bf16 during the HBM load (SWDGE cast-DMA) into
their final layouts. mm1 emission is interleaved with the LIF scan so the
tensor engine stream k0,k1,ms0-7,k2,ms8-15,k3,ms16-31 stays dense and warm.
"""

import numpy as np
from contextlib import ExitStack

import concourse.bass as bass
import concourse.tile as tile
from concourse import bacc, mybir
from concourse.bass_utils import run_bass_kernel_spmd

N_CORES = 8


def _lif_op():
    """Fused LIF step as a custom DVE op:
         out = select(in0 < s0, in0, 0) * s1 + in1
       i.e. w_new = reset(w_old)*0.5 + hh  in a single VectorE instruction."""
    from concourse import dve_ops
    from concourse.dve_spec import Spec, Src0, Src1, Zero, C0, C1, select, lower
    from concourse.dve_uop import DveOpSpec

    for op in dve_ops.OPS:
        if op.name == "LIF_STEP_ANT":
            return op
    spec = Spec(
        body=select(Src0 < C0, Src0, Zero) * C1 + Src1,
        reference=lambda in0, in1, s0, s1, imm2: (
            np.where(in0.astype(np.float32) < s0, in0.astype(np.float32), 0.0) * s1
            + in1.astype(np.float32)).astype(np.float32),
    )
    row = dve_ops._CUSTOM_DVE_ROW_BASE + len(dve_ops.OPS)
    shas = {}
    for ver in ("v3", "v4"):
        try:
            shas[ver] = DveOpSpec(name="LIF_STEP_ANT", opcode=row,
                                  uops=lower(spec, ver=ver), rd1_en=True).sha(ver)
        except Exception:
            pass
    op = dve_ops.DveOp("LIF_STEP_ANT", spec, subdim=False, uops_sha=shas)
    dve_ops.OPS.append(op)
    dve_ops._SUB_OPCODE_FOR_NAME[op.name] = row
    dve_ops.CUSTOM_DVE_SPECS[op.name] = spec
    return op
T, B, D, C = 32, 512, 1024, 1000
BC = B // N_CORES          # 64 rows per core
TB = T * BC                # 2048 matmul rows per core
FP32 = mybir.dt.float32
BF16 = mybir.dt.bfloat16
FP8 = mybir.dt.float8e4
W1_PRESCALE = 256.0   # host multiplies W1 by this (exact power of 2) so its
                      # small uniform(-1/32,1/32) values stay in fp8e4m3's
                      # normal range; compensated in the h-copy scale
AF = mybir.ActivationFunctionType
OP = mybir.AluOpType


def build_program():
    nc = bacc.Bacc("TRN2", target_bir_lowering=False, debug=False, num_devices=N_CORES)

    xt_d = nc.dram_tensor("xT", [D, TB], FP8, kind="ExternalInput").ap()
    w1t_d = nc.dram_tensor("W1T", [D, D], FP8, kind="ExternalInput").ap()
    b1_d = nc.dram_tensor("b1", [D], FP32, kind="ExternalInput").ap()
    w2t_d = nc.dram_tensor("W2T", [D, C], FP8, kind="ExternalInput").ap()
    b2_d = nc.dram_tensor("b2", [C], FP32, kind="ExternalInput").ap()
    y_d = nc.dram_tensor("y", [BC, C], FP32, kind="ExternalOutput").ap()

    with tile.TileContext(nc) as tc, ExitStack() as ctx:
        persist = ctx.enter_context(tc.tile_pool(name="persist", bufs=1))
        mpool = ctx.enter_context(tc.tile_pool(name="mpool", bufs=T))
        small = ctx.enter_context(tc.tile_pool(name="small", bufs=1))
        ps_h = ctx.enter_context(tc.tile_pool(name="ps_h", bufs=4, space="PSUM"))
        ps_ms = ctx.enter_context(tc.tile_pool(name="ps_ms", bufs=1, space="PSUM"))
        ps_y = ctx.enter_context(tc.tile_pool(name="ps_y", bufs=2, space="PSUM"))

        # ---- constants / biases needed early ----
        io = small.tile([128, 128], mybir.dt.int32)
        nc.gpsimd.iota(io[:], pattern=[[1, 128]], base=0, channel_multiplier=-1)
        ident = small.tile([128, 128], BF16)
        nc.vector.tensor_scalar(ident[:], io[:], 0, None, op0=OP.is_equal)

        b1_sb = small.tile([128, 8], FP32)
        nc.scalar.dma_start(b1_sb[:], b1_d.rearrange("(j p) -> p j", p=128))
        b1h = small.tile([128, 8], FP32)
        nc.vector.tensor_scalar_mul(b1h[:], b1_sb[:], 0.5)

        # warm the ACT Exp/Ln spline tables during the prologue so the
        # epilogue doesn't pay the ~2.7us table-load switch
        warm = small.tile([1, 8], FP32)
        nc.scalar.activation(warm[:, 0:4], b1_sb[0:1, 0:4], AF.Exp)
        nc.scalar.activation(warm[:, 4:8], b1_sb[0:1, 4:8], AF.Ln)

        # ---- W1T / xT: direct bf16 loads, split across the two HWDGE rings
        # (sync + scalar). Each ring drains strictly in FIFO order, so issuing
        # W1T, x0, x1, x2, x3, W2T on both rings gives exact load priority. ----
        w1t = persist.tile([128, 8 * 1024], FP8)
        w1t3 = w1t[:].rearrange("p (j e) -> p j e", j=8)
        w1src = w1t_d[:].rearrange("(dj p) e -> p dj e", p=128)
        nc.sync.dma_start(w1t3[:, 0:4, :], w1src[:, 0:4, :])
        nc.scalar.dma_start(w1t3[:, 4:8, :], w1src[:, 4:8, :])

        xt = persist.tile([128, 8 * TB], FP8)
        xt3 = xt[:].rearrange("p (j t) -> p j t", j=8)

        def load_x_chunk(k):
            src_k = xt_d[:, k * 512:(k + 1) * 512].rearrange("(dj p) t -> p dj t", p=128)
            nc.sync.dma_start(xt3[:, 0:4, k * 512:(k + 1) * 512], src_k[:, 0:4, :])
            nc.scalar.dma_start(xt3[:, 4:8, k * 512:(k + 1) * 512], src_k[:, 4:8, :])

        load_x_chunk(0)
        load_x_chunk(1)

        # ---- matmul1: h[e, tb] = W1 @ x.T, fused 0.5*h + 0.5*b1 into scan layout ----
        # h_sb free index = t*512 + j*64 + b
        h_sb = persist.tile([128, T * 512], BF16)
        h3 = h_sb[:].rearrange("p (t x) -> p t x", x=512)

        def mm1_group(g, t0, tcnt):
            n = tcnt * 64
            for j in range(8):
                ps = ps_h.tile([128, 512], FP32, tag="ps_h", name=f"psh_{g}_{j}")
                for dp in range(4):   # pairs of contraction tiles (DoubleRow)
                    nc.tensor.matmul(
                        ps[:, 0:n],
                        w1t3[:, 2 * dp:2 * dp + 2, j * 128:(j + 1) * 128],
                        xt3[:, 2 * dp:2 * dp + 2, t0 * 64:(t0 + tcnt) * 64],
                        start=(dp == 0), stop=(dp == 3),
                        perf_mode=mybir.MatmulPerfMode.DoubleRow,
                    )
                nc.scalar.activation(
                    h3[:, t0:t0 + tcnt, j * 64:(j + 1) * 64],
                    ps[:, 0:n].rearrange("p (t b) -> p t b", t=tcnt),
                    AF.Identity, scale=0.5 / W1_PRESCALE, bias=b1h[:, j:j + 1],
                )

        # ---- LIF scan pieces (emitted interleaved with mm1 groups) ----
        # state wst = pre-reset membrane voltage v'_t; one fused DVE op per step:
        #   wst = select(wst < 1, wst, 0) * 0.5 + hh_t
        lif = _lif_op()
        wst = small.tile([128, 512], BF16)
        nc.vector.memset(wst[:], 0.0)
        msum = ps_ms.tile([128, 512], FP32)

        def scan_steps(t0, t1):
            for t in range(t0, t1):
                h_t = h_sb[:, t * 512:(t + 1) * 512]
                nc.vector._custom_dve(lif, out=wst[:], in0=wst[:], in1=h_t,
                                      s0=1.0, s1=0.5)
                m = mpool.tile([128, 512], BF16, tag="m", name=f"m{t}")
                nc.vector.tensor_scalar(m[:], wst[:], 1.0, None, op0=OP.is_lt)
                nc.tensor.matmul(msum[:], ident[:], m[:],
                                 start=(t == 0), stop=(t == T - 1))

        load_x_chunk(2)
        load_x_chunk(3)
        # ---- W2T (queued last on the rings): direct bf16 load ----
        w2t = persist.tile([128, 8 * 1024], FP8)
        w2t3 = w2t[:].rearrange("p (j c) -> p j c", j=8)
        w2src = w2t_d[:].rearrange("(ej p) c -> p ej c", p=128)
        nc.sync.dma_start(w2t3[:, 0:4, 0:C], w2src[:, 0:4, :])
        nc.scalar.dma_start(w2t3[:, 4:8, 0:C], w2src[:, 4:8, :])
        mm1_group(0, 0, 4)
        mm1_group(1, 4, 8)
        scan_steps(0, 4)
        mm1_group(2, 12, 8)
        scan_steps(4, 12)
        mm1_group(3, 20, 8)
        scan_steps(12, 20)
        mm1_group(4, 28, 4)
        scan_steps(20, 32)

        # sum_t s_t = T - msum; spike counts are small integers, exact in fp8e4m3
        ssum = small.tile([128, 512], FP8)
        nc.scalar.activation(ssum[:], msum[:], AF.Copy, scale=-1.0, bias=float(T))
        ssum3 = ssum[:].rearrange("p (j b) -> p j b", j=8)


        b2_sb = small.tile([1, C], FP32)
        nc.scalar.dma_start(b2_sb[:], b2_d.rearrange("(a c) -> a c", a=1))
        b2_32 = small.tile([1, C], BF16)
        nc.scalar.activation(b2_32[:], b2_sb[:], AF.Copy,
                             scale=float(T) * W1_PRESCALE)
        ones = small.tile([1, BC], BF16)
        nc.vector.memset(ones[:], 1.0)

        # ---- matmul2: y = ssum @ W2.T / T + b2 ----
        y_sb = small.tile([BC, 1024], FP32)
        for half in range(2):
            n = 512 if half == 0 else C - 512
            c0 = half * 512
            psy = ps_y.tile([BC, 512], FP32, tag="ps_y", name=f"psy{half}")
            for pj in range(4):   # DoubleRow pairs of e-tiles
                nc.tensor.matmul(
                    psy[:, 0:n],
                    ssum3[:, 2 * pj:2 * pj + 2, :],
                    w2t3[:, 2 * pj:2 * pj + 2, c0:c0 + n],
                    start=(pj == 0), stop=False,
                    perf_mode=mybir.MatmulPerfMode.DoubleRow,
                )
            nc.tensor.matmul(psy[:, 0:n], ones[:], b2_32[:, c0:c0 + n],
                             start=False, stop=True)
            nc.scalar.activation(y_sb[:, c0:c0 + n], psy[:, 0:n], AF.Copy,
                                 scale=1.0 / (T * W1_PRESCALE))

        # ---- log_softmax over C (y is small enough that no max-shift is needed:
        # |y| <= D/T + |b2| ~ 32, exp stays in fp32 range) ----
        ez = small.tile([BC, 1024], FP32)
        nc.scalar.activation(ez[:, 0:C], y_sb[:, 0:C], AF.Exp)
        ssum_e = small.tile([BC, 1], FP32)
        nc.vector.reduce_sum(ssum_e[:], ez[:, 0:C], axis=mybir.AxisListType.X)
        lse = small.tile([BC, 1], FP32)
        nc.scalar.activation(lse[:], ssum_e[:], AF.Ln)
        out_sb = small.tile([BC, C], FP32)
        nc.vector.tensor_scalar(out_sb[:], y_sb[:, 0:C], lse[:], None, op0=OP.subtract)
        nc.sync.dma_start(y_d[:], out_sb[:])

    nc.compile()
    return nc


_CACHE = {}


def kernel(x, W1, b1, W2, b2):
    if "nc" not in _CACHE:
        _CACHE["nc"] = build_program()
    nc = _CACHE["nc"]

    import ml_dtypes
    f8 = mybir.dt.np(FP8)
    x = np.asarray(x, dtype=np.float32)
    w1t = np.ascontiguousarray(
        (np.asarray(W1, dtype=np.float32).T * W1_PRESCALE).astype(f8))
    w2t = np.ascontiguousarray(
        (np.asarray(W2, dtype=np.float32).T * W1_PRESCALE).astype(f8))
    b1 = np.ascontiguousarray(b1, dtype=np.float32)
    b2 = np.ascontiguousarray(b2, dtype=np.float32)
    in_maps = []
    for i in range(N_CORES):
        xs = np.ascontiguousarray(
            x[:, i * BC:(i + 1) * BC, :].reshape(TB, D).T.astype(f8))
        in_maps.append({"xT": xs, "W1T": w1t, "b1": b1, "W2T": w2t, "b2": b2})

    res = run_bass_kernel_spmd(nc, in_maps, core_ids=list(range(N_CORES)),
                               **_CACHE.get("run_kwargs", {}))
    _CACHE["last_results"] = res
    out = np.concatenate([res.results[i]["y"] for i in range(N_CORES)], axis=0)
    return out



# revision 2
# speedup vs baseline: 1.0248x; 1.0248x over previous
"""Trainium2 Bass kernel for nn_CLFBlock (linear -> LIF scan -> linear -> T-mean -> log_softmax).

Self-contained: hardcodes shapes T=32, B=512, D=1024, C=1000 and data-parallel
sharding of the batch dim across 8 NeuronCores.

v3 schedule (ops identical to the proven baseline mix, pipeline rebuilt):
  - Host lays W1T out in j-major 128KB blocks and b1h/b2 pre-packed, so the
    DMA rings deliver exactly what the first matmul group needs first; the
    first mm1 MM issues ~1.5us after DMA flow starts instead of ~7us.
  - Junk matmuls (no data deps) run during the DMA fill to flip the PE HAM
    clock gate to 2.4GHz before real work arrives.
  - mm1 groups of tcnt=[4,4,8,8,8] timesteps: per-(g,j) PSUM tiles (1 bank,
    4-deep rotation), evicted to h_sb by Scalar ACT (scale 0.5/PRESCALE +
    0.5*b1 bias) right after each j's 4 DoubleRow MMs -> h streams into SBUF
    at 0.87us/t while the DVE scan consumes at ~1.03us/t, so the chain never
    starves and mm1 stays at the fp8-DoubleRow roofline.
  - LIF scan: fused custom DVE op (state update) + is_lt mask per step; spike
    sums accumulate on the PE (ident @ mask), deferred a few steps so mask
    waits never sit ahead of mm1 in the PE FIFO; junk MMs keep HAM warm
    through the scan tail so mm2 runs at full clock.
  - Epilogue: Exp table pre-warmed while the scan drains; y copies on the
    (idle) Vector engine; log-softmax; output DMA split in halves.
"""

import numpy as np
from contextlib import ExitStack

import concourse.bass as bass
import concourse.tile as tile
from concourse import bacc, mybir
from concourse.bass_utils import run_bass_kernel_spmd

N_CORES = 8
T, B, D, C = 32, 512, 1024, 1000
BC = B // N_CORES          # 64 rows per core
TB = T * BC                # 2048 matmul rows per core
FP32 = mybir.dt.float32
BF16 = mybir.dt.bfloat16
FP8 = mybir.dt.float8e4
W1_PRESCALE = 256.0   # host multiplies W1/W2 by this (exact power of 2) so
                      # their small uniform(-1/32,1/32) values stay in
                      # fp8e4m3's normal range; compensated on-chip
AF = mybir.ActivationFunctionType
OP = mybir.AluOpType

TCNTS = [4, 4, 8, 8, 8]
OFFS = [0, 4, 8, 16, 24, 32]


def _lif_op():
    """Fused LIF step: out = select(in0 < s0, in0, 0)*s1 + in1."""
    from concourse import dve_ops
    from concourse.dve_spec import Spec, Src0, Src1, Zero, C0, C1, select, lower
    from concourse.dve_uop import DveOpSpec

    for op in dve_ops.OPS:
        if op.name == "LIF_STEP_ANT":
            return op
    spec = Spec(
        body=select(Src0 < C0, Src0, Zero) * C1 + Src1,
        reference=lambda in0, in1, s0, s1, imm2: (
            np.where(in0.astype(np.float32) < s0, in0.astype(np.float32), 0.0) * s1
            + in1.astype(np.float32)).astype(np.float32),
    )
    row = dve_ops._CUSTOM_DVE_ROW_BASE + len(dve_ops.OPS)
    shas = {}
    for ver in ("v3", "v4"):
        try:
            shas[ver] = DveOpSpec(name="LIF_STEP_ANT", opcode=row,
                                  uops=lower(spec, ver=ver), rd1_en=True).sha(ver)
        except Exception:
            pass
    op = dve_ops.DveOp("LIF_STEP_ANT", spec, subdim=False, uops_sha=shas)
    dve_ops.OPS.append(op)
    dve_ops._SUB_OPCODE_FOR_NAME[op.name] = row
    dve_ops.CUSTOM_DVE_SPECS[op.name] = spec
    return op


def build_program():
    nc = bacc.Bacc("TRN2", target_bir_lowering=False, debug=False, num_devices=N_CORES)

    # W1T in j-major blocks: host layout [j=8][p=128][dj=8][e=128] so each
    # block is a contiguous 128KB transfer with 1KB rows
    w1j_d = nc.dram_tensor("W1J", [8, 128, 8 * 128], FP8, kind="ExternalInput").ap()
    xt_d = nc.dram_tensor("xT", [D, TB], FP8, kind="ExternalInput").ap()
    w2t_d = nc.dram_tensor("W2T", [D, C], FP8, kind="ExternalInput").ap()
    b1h_d = nc.dram_tensor("b1h", [128, 8], FP32, kind="ExternalInput").ap()
    b2_d = nc.dram_tensor("b2", [C], FP32, kind="ExternalInput").ap()
    y_d = nc.dram_tensor("y", [BC, C], FP32, kind="ExternalOutput").ap()

    lif = _lif_op()

    with tile.TileContext(nc) as tc, ExitStack() as ctx:
        sb = ctx.enter_context(tc.tile_pool(name="sb", bufs=1))
        mpool = ctx.enter_context(tc.tile_pool(name="mpool", bufs=6))

        # ---- constants with no DMA deps (warmup operands, scan state) ----
        io = sb.tile([128, 128], mybir.dt.int32)
        nc.gpsimd.iota(io[:], pattern=[[1, 128]], base=0, channel_multiplier=-1)
        ident = sb.tile([128, 128], BF16)
        nc.vector.tensor_scalar(ident[:], io[:], 0, None, op0=OP.is_equal)
        junkd = sb.tile([128, 512], BF16)
        nc.vector.memset(junkd[:], 0.125)
        wroll = sb.tile([128, 4 * 512], BF16)
        nc.vector.memset(wroll[:, 3 * 512:4 * 512], 0.0)   # w_{-1} = 0 in buf 3

        # ---- input DMA (two HWDGE rings, FIFO priority order):
        # b1h first (tiny, needed by the first eviction), then x cols 0-255,
        # then the 8 W1 j-blocks, then the rest of x, W2, b2 ----
        # w1t4: [p, j, dj, e]; one contiguous half-of-W1 transfer per ring
        w1t = sb.tile([128, 8 * 1024], FP8)
        w1t4 = w1t[:].rearrange("p (j k e) -> p j k e", j=8, k=8)
        w1thirds = w1t[:].rearrange("p (j x) -> p j x", j=8)
        w1src = w1j_d[:].rearrange("j p x -> p j x")
        nc.sync.dma_start(w1thirds[:, 0:4, :], w1src[:, 0:4, :])
        nc.scalar.dma_start(w1thirds[:, 4:8, :], w1src[:, 4:8, :])

        b1h = sb.tile([128, 8], FP32)
        nc.scalar.dma_start(b1h[:], b1h_d)

        xt = sb.tile([128, 8 * TB], FP8)
        xt3 = xt[:].rearrange("p (j t) -> p j t", j=8)

        def load_x_cols(c0, c1):
            src = xt_d[:, c0:c1].rearrange("(dj p) t -> p dj t", p=128)
            nc.sync.dma_start(xt3[:, 0:4, c0:c1], src[:, 0:4, :])
            nc.scalar.dma_start(xt3[:, 4:8, c0:c1], src[:, 4:8, :])

        load_x_cols(0, 512)
        load_x_cols(512, 1024)
        load_x_cols(1024, 1536)
        load_x_cols(1536, 2048)

        w2t = sb.tile([128, 8 * 1024], FP8)
        w2t3 = w2t[:].rearrange("p (j c) -> p j c", j=8)
        w2src = w2t_d[:].rearrange("(ej p) c -> p ej c", p=128)
        nc.sync.dma_start(w2t3[:, 0:4, 0:C], w2src[:, 0:4, :])
        nc.scalar.dma_start(w2t3[:, 4:8, 0:C], w2src[:, 4:8, :])
        b2_sb = sb.tile([1, C], FP32)
        nc.sync.dma_start(b2_sb[:], b2_d.rearrange("(a c) -> a c", a=1))

        # ---- persistent SBUF state ----
        h_sb = sb.tile([128, T * 512], BF16)
        h3 = h_sb[:].rearrange("p (t x) -> p t x", x=512)
        ssum = sb.tile([128, 512], FP8)
        y_sb = sb.tile([BC, 1024], FP32)

        with tc.tile_pool(name="ps_h", bufs=4, space="PSUM") as ps_h, \
             tc.tile_pool(name="psm", bufs=1, space="PSUM") as psm:

            msum = psm.tile([128, 512], FP32)
            # HAM warm-up during the DMA fill: tiny N=16 matmuls keep the PE
            # "busy" for the activity monitor at ~3% of a full MM's power (a
            # dense warmup measurably trips the chip-wide P0 power throttle
            # and slows every engine ~17%); msum's first real MM clears
            for i in range(55):
                nc.tensor.matmul(msum[:, 0:16], ident[:], junkd[:, 0:16],
                                 start=True, stop=True)

            n_msum = [0]
            masks = {}

            def emit_msum(t):
                m = masks.pop(t)
                nc.tensor.matmul(msum[:], ident[:], m[:],
                                 start=(n_msum[0] == 0), stop=(n_msum[0] == T - 1))
                n_msum[0] += 1

            def emit_group(g):
                t0, tcnt = OFFS[g], TCNTS[g]
                n = tcnt * 64
                for j in range(8):
                    ps = ps_h.tile([128, 512], FP32, tag="ps_h", name=f"ps_{g}_{j}")
                    for dp in range(4):
                        nc.tensor.matmul(
                            ps[:, 0:n],
                            w1t4[:, j, 2 * dp:2 * dp + 2, :],
                            xt3[:, 2 * dp:2 * dp + 2, t0 * 64:(t0 + tcnt) * 64],
                            start=(dp == 0), stop=(dp == 3),
                            perf_mode=mybir.MatmulPerfMode.DoubleRow,
                        )
                    nc.scalar.activation(
                        h3[:, t0:t0 + tcnt, j * 64:(j + 1) * 64],
                        ps[:, 0:n].rearrange("p (t b) -> p t b", t=tcnt),
                        AF.Identity, scale=0.5 / W1_PRESCALE, bias=b1h[:, j:j + 1],
                    )

            def emit_scan(t0, t1):
                for t in range(t0, t1):
                    a = wroll[:, ((t - 1) % 4) * 512:((t - 1) % 4) * 512 + 512]
                    b_ = wroll[:, (t % 4) * 512:(t % 4) * 512 + 512]
                    nc.vector._custom_dve(lif, out=b_, in0=a,
                                          in1=h_sb[:, t * 512:(t + 1) * 512],
                                          s0=1.0, s1=0.5)
                    m = mpool.tile([128, 512], BF16, tag="m", name=f"m{t}")
                    nc.vector.tensor_scalar(m[:], b_, 1.0, None, op0=OP.is_lt)
                    masks[t] = m

            emit_group(0)
            emit_scan(0, 4)
            emit_group(1)
            emit_scan(4, 8)
            emit_group(2)
            emit_scan(8, 16)
            for t in range(0, 4):
                emit_msum(t)
            emit_group(3)
            emit_scan(16, 24)
            for t in range(4, 12):
                emit_msum(t)
            emit_group(4)
            emit_scan(24, 32)
            for t in range(12, 20):
                emit_msum(t)
            # scan tail: alternate spike-sum MMs with junk MMs (into a retired
            # ps_h buffer) to keep the PE HAM clock warm into mm2
            for t in range(20, 32):
                emit_msum(t)
            # warm the Exp spline table while the scan drains (Scalar is idle
            # once the last eviction is done)
            nc.scalar.activation(y_sb[0:1, 0:4], b2_sb[0:1, 0:4], AF.Exp)
            # ssum = T - msum (spike counts are small ints, exact in fp8)
            nc.scalar.activation(ssum[:], msum[:], AF.Copy, scale=-1.0,
                                 bias=float(T))

        ssum3 = ssum[:].rearrange("p (j b) -> p j b", j=8)
        b2_32 = sb.tile([1, C], BF16)
        nc.scalar.activation(b2_32[:], b2_sb[:], AF.Copy,
                             scale=float(T) * W1_PRESCALE)
        ones = sb.tile([1, BC], BF16)
        nc.vector.memset(ones[:], 1.0)

        with tc.tile_pool(name="psy", bufs=2, space="PSUM") as psy_pool:
            for half in range(2):
                n = 512 if half == 0 else C - 512
                c0 = half * 512
                psy = psy_pool.tile([BC, 512], FP32, tag="ps_y", name=f"psy{half}")
                for pj in range(4):
                    nc.tensor.matmul(
                        psy[:, 0:n],
                        ssum3[:, 2 * pj:2 * pj + 2, :],
                        w2t3[:, 2 * pj:2 * pj + 2, c0:c0 + n],
                        start=(pj == 0), stop=False,
                        perf_mode=mybir.MatmulPerfMode.DoubleRow,
                    )
                nc.tensor.matmul(psy[:, 0:n], ones[:], b2_32[:, c0:c0 + n],
                                 start=False, stop=True)
                nc.vector.tensor_scalar(y_sb[:, c0:c0 + n], psy[:, 0:n],
                                        1.0 / (float(T) * W1_PRESCALE), None,
                                        op0=OP.mult)

        # ---- log_softmax over C (|y| small enough that no max-shift needed) ----
        ez = sb.tile([BC, 1024], FP32)
        nc.scalar.activation(ez[:, 0:C], y_sb[:, 0:C], AF.Exp)
        ssum_e = sb.tile([BC, 1], FP32)
        nc.vector.reduce_sum(ssum_e[:], ez[:, 0:C], axis=mybir.AxisListType.X)
        lse = sb.tile([BC, 1], FP32)
        nc.scalar.activation(lse[:], ssum_e[:], AF.Ln)
        out_sb = sb.tile([BC, C], FP32)
        nc.vector.tensor_scalar(out_sb[:, 0:512], y_sb[:, 0:512], lse[:], None,
                                op0=OP.subtract)
        nc.sync.dma_start(y_d[:, 0:512], out_sb[:, 0:512])
        nc.vector.tensor_scalar(out_sb[:, 512:C], y_sb[:, 512:C], lse[:], None,
                                op0=OP.subtract)
        nc.sync.dma_start(y_d[:, 512:C], out_sb[:, 512:C])

    nc.compile()
    return nc


_CACHE = {}


def kernel(x, W1, b1, W2, b2):
    if "nc" not in _CACHE:
        _CACHE["nc"] = build_program()
    nc = _CACHE["nc"]

    f8 = mybir.dt.np(FP8)
    x = np.asarray(x, dtype=np.float32)
    w1t = (np.asarray(W1, dtype=np.float32).T * W1_PRESCALE).astype(f8)
    # j-major blocks: [j, p, dj, e] from W1T [dj*128+p, j*128+e]
    w1j = np.ascontiguousarray(
        w1t.reshape(8, 128, 8, 128).transpose(2, 1, 0, 3)).reshape(8, 128, 8 * 128)
    w2t = np.ascontiguousarray(
        (np.asarray(W2, dtype=np.float32).T * W1_PRESCALE).astype(f8))
    b1h = np.ascontiguousarray(
        (0.5 * np.asarray(b1, dtype=np.float32)).reshape(8, 128).T)
    b2 = np.ascontiguousarray(b2, dtype=np.float32)
    in_maps = []
    for i in range(N_CORES):
        xs = np.ascontiguousarray(
            x[:, i * BC:(i + 1) * BC, :].reshape(TB, D).T.astype(f8))
        in_maps.append({"xT": xs, "W1J": w1j, "W2T": w2t, "b1h": b1h, "b2": b2})

    res = run_bass_kernel_spmd(nc, in_maps, core_ids=list(range(N_CORES)),
                               **_CACHE.get("run_kwargs", {}))
    _CACHE["last_results"] = res
    out = np.concatenate([res.results[i]["y"] for i in range(N_CORES)], axis=0)
    return out


# revision 3
# speedup vs baseline: 1.0424x; 1.0172x over previous
"""Trainium2 Bass kernel for nn_CLFBlock (linear -> LIF scan -> linear -> T-mean -> log_softmax).

Self-contained: hardcodes shapes T=32, B=512, D=1024, C=1000 and data-parallel
sharding of the batch dim across 8 NeuronCores.

v3 schedule (ops identical to the proven baseline mix, pipeline rebuilt):
  - Host lays W1T out in j-major 128KB blocks and b1h/b2 pre-packed, so the
    DMA rings deliver exactly what the first matmul group needs first; the
    first mm1 MM issues ~1.5us after DMA flow starts instead of ~7us.
  - Junk matmuls (no data deps) run during the DMA fill to flip the PE HAM
    clock gate to 2.4GHz before real work arrives.
  - mm1 groups of tcnt=[4,4,8,8,8] timesteps: per-(g,j) PSUM tiles (1 bank,
    4-deep rotation), evicted to h_sb by Scalar ACT (scale 0.5/PRESCALE +
    0.5*b1 bias) right after each j's 4 DoubleRow MMs -> h streams into SBUF
    at 0.87us/t while the DVE scan consumes at ~1.03us/t, so the chain never
    starves and mm1 stays at the fp8-DoubleRow roofline.
  - LIF scan: fused custom DVE op (state update) + is_lt mask per step; spike
    sums accumulate on the PE (ident @ mask), deferred a few steps so mask
    waits never sit ahead of mm1 in the PE FIFO; junk MMs keep HAM warm
    through the scan tail so mm2 runs at full clock.
  - Epilogue: Exp table pre-warmed while the scan drains; y copies on the
    (idle) Vector engine; log-softmax; output DMA split in halves.
"""

import numpy as np
from contextlib import ExitStack

import concourse.bass as bass
import concourse.tile as tile
from concourse import bacc, mybir
from concourse.bass_utils import run_bass_kernel_spmd

N_CORES = 8
T, B, D, C = 32, 512, 1024, 1000
BC = B // N_CORES          # 64 rows per core
TB = T * BC                # 2048 matmul rows per core
FP32 = mybir.dt.float32
BF16 = mybir.dt.bfloat16
FP8 = mybir.dt.float8e4
W1_PRESCALE = 256.0   # host multiplies W1/W2 by this (exact power of 2) so
                      # their small uniform(-1/32,1/32) values stay in
                      # fp8e4m3's normal range; compensated on-chip
AF = mybir.ActivationFunctionType
OP = mybir.AluOpType

TCNTS = [4, 4, 6, 6, 6, 6]
OFFS = [0, 4, 8, 14, 20, 26, 32]


def _lif_op():
    """Fused LIF step: out = select(in0 < s0, in0, 0)*s1 + in1."""
    from concourse import dve_ops
    from concourse.dve_spec import Spec, Src0, Src1, Zero, C0, C1, select, lower
    from concourse.dve_uop import DveOpSpec

    for op in dve_ops.OPS:
        if op.name == "LIF_STEP_ANT":
            return op
    spec = Spec(
        body=select(Src0 < C0, Src0, Zero) * C1 + Src1,
        reference=lambda in0, in1, s0, s1, imm2: (
            np.where(in0.astype(np.float32) < s0, in0.astype(np.float32), 0.0) * s1
            + in1.astype(np.float32)).astype(np.float32),
    )
    row = dve_ops._CUSTOM_DVE_ROW_BASE + len(dve_ops.OPS)
    shas = {}
    for ver in ("v3", "v4"):
        try:
            shas[ver] = DveOpSpec(name="LIF_STEP_ANT", opcode=row,
                                  uops=lower(spec, ver=ver), rd1_en=True).sha(ver)
        except Exception:
            pass
    op = dve_ops.DveOp("LIF_STEP_ANT", spec, subdim=False, uops_sha=shas)
    dve_ops.OPS.append(op)
    dve_ops._SUB_OPCODE_FOR_NAME[op.name] = row
    dve_ops.CUSTOM_DVE_SPECS[op.name] = spec
    return op


def build_program():
    nc = bacc.Bacc("TRN2", target_bir_lowering=False, debug=False, num_devices=N_CORES)

    # W1T in j-major blocks: host layout [j=8][p=128][dj=8][e=128] so each
    # block is a contiguous 128KB transfer with 1KB rows
    w1j_d = nc.dram_tensor("W1J", [8, 128, 8 * 128], FP8, kind="ExternalInput").ap()
    xt_d = nc.dram_tensor("xT", [D, TB], FP8, kind="ExternalInput").ap()
    w2t_d = nc.dram_tensor("W2T", [D, C], FP8, kind="ExternalInput").ap()
    b1h_d = nc.dram_tensor("b1h", [128, 8], FP32, kind="ExternalInput").ap()
    b2_d = nc.dram_tensor("b2", [C], FP32, kind="ExternalInput").ap()
    y_d = nc.dram_tensor("y", [BC, C], FP32, kind="ExternalOutput").ap()

    lif = _lif_op()

    with tile.TileContext(nc) as tc, ExitStack() as ctx:
        sb = ctx.enter_context(tc.tile_pool(name="sb", bufs=1))
        mpool = ctx.enter_context(tc.tile_pool(name="mpool", bufs=10))

        # ---- constants with no DMA deps (warmup operands, scan state) ----
        io = sb.tile([128, 128], mybir.dt.int32)
        nc.gpsimd.iota(io[:], pattern=[[1, 128]], base=0, channel_multiplier=-1)
        ident = sb.tile([128, 128], BF16)
        nc.vector.tensor_scalar(ident[:], io[:], 0, None, op0=OP.is_equal)
        junkd = sb.tile([128, 512], BF16)
        nc.vector.memset(junkd[:], 0.125)
        wroll = sb.tile([128, 4 * 512], BF16)
        nc.vector.memset(wroll[:, 3 * 512:4 * 512], 0.0)   # w_{-1} = 0 in buf 3

        # ---- input DMA (two HWDGE rings, FIFO priority order):
        # b1h first (tiny, needed by the first eviction), then x cols 0-255,
        # then the 8 W1 j-blocks, then the rest of x, W2, b2 ----
        # w1t4: [p, j, dj, e]; one contiguous half-of-W1 transfer per ring
        w1t = sb.tile([128, 8 * 1024], FP8)
        w1t4 = w1t[:].rearrange("p (j k e) -> p j k e", j=8, k=8)
        w1thirds = w1t[:].rearrange("p (j x) -> p j x", j=8)
        w1src = w1j_d[:].rearrange("j p x -> p j x")
        # sync ring issues ~2us earlier than scalar: give it 6 of 8 W1 blocks
        nc.sync.dma_start(w1thirds[:, 0:6, :], w1src[:, 0:6, :])
        nc.scalar.dma_start(w1thirds[:, 6:8, :], w1src[:, 6:8, :])

        b1h = sb.tile([128, 8], FP32)
        nc.scalar.dma_start(b1h[:], b1h_d)

        xt = sb.tile([128, 8 * TB], FP8)
        xt3 = xt[:].rearrange("p (j t) -> p j t", j=8)
        xsrc = xt_d[:].rearrange("(dj p) t -> p dj t", p=128)

        # x cols 0-511 ride the scalar ring (idle after its 2 W1 blocks)
        nc.scalar.dma_start(xt3[:, 0:4, 0:512], xsrc[:, 0:4, 0:512])
        nc.scalar.dma_start(xt3[:, 4:8, 0:512], xsrc[:, 4:8, 0:512])

        def load_x_cols(c0, c1):
            nc.sync.dma_start(xt3[:, 0:4, c0:c1], xsrc[:, 0:4, c0:c1])
            nc.scalar.dma_start(xt3[:, 4:8, c0:c1], xsrc[:, 4:8, c0:c1])

        load_x_cols(512, 1024)
        load_x_cols(1024, 1536)
        load_x_cols(1536, 2048)

        w2t = sb.tile([128, 8 * 1024], FP8)
        w2t3 = w2t[:].rearrange("p (j c) -> p j c", j=8)
        w2src = w2t_d[:].rearrange("(ej p) c -> p ej c", p=128)
        nc.sync.dma_start(w2t3[:, 0:4, 0:C], w2src[:, 0:4, :])
        nc.scalar.dma_start(w2t3[:, 4:8, 0:C], w2src[:, 4:8, :])
        b2_sb = sb.tile([1, C], FP32)
        nc.sync.dma_start(b2_sb[:], b2_d.rearrange("(a c) -> a c", a=1))

        # ---- persistent SBUF state ----
        h_sb = sb.tile([128, T * 512], BF16)
        h3 = h_sb[:].rearrange("p (t x) -> p t x", x=512)
        ssum = sb.tile([128, 512], FP8)
        y_sb = sb.tile([BC, 1024], FP32)
        ez = sb.tile([BC, 1024], FP32)

        with tc.tile_pool(name="ps_h", bufs=4, space="PSUM") as ps_h, \
             tc.tile_pool(name="psm", bufs=1, space="PSUM") as psm:

            msum = psm.tile([128, 512], FP32)
            # HAM warm-up during the DMA fill: tiny N=16 matmuls keep the PE
            # "busy" for the activity monitor at ~3% of a full MM's power (a
            # dense warmup measurably trips the chip-wide P0 power throttle
            # and slows every engine ~17%); msum's first real MM clears
            for i in range(55):
                nc.tensor.matmul(msum[:, 0:16], ident[:], junkd[:, 0:16],
                                 start=True, stop=True)

            n_msum = [0]
            masks = {}

            def emit_msum(t):
                m = masks.pop(t)
                nc.tensor.matmul(msum[:], ident[:], m[:],
                                 start=(n_msum[0] == 0), stop=(n_msum[0] == T - 1))
                n_msum[0] += 1

            def emit_group(g):
                t0, tcnt = OFFS[g], TCNTS[g]
                n = tcnt * 64
                for j in range(8):
                    ps = ps_h.tile([128, 512], FP32, tag="ps_h", name=f"ps_{g}_{j}")
                    for dp in range(4):
                        nc.tensor.matmul(
                            ps[:, 0:n],
                            w1t4[:, j, 2 * dp:2 * dp + 2, :],
                            xt3[:, 2 * dp:2 * dp + 2, t0 * 64:(t0 + tcnt) * 64],
                            start=(dp == 0), stop=(dp == 3),
                            perf_mode=mybir.MatmulPerfMode.DoubleRow,
                        )
                    nc.scalar.activation(
                        h3[:, t0:t0 + tcnt, j * 64:(j + 1) * 64],
                        ps[:, 0:n].rearrange("p (t b) -> p t b", t=tcnt),
                        AF.Identity, scale=0.5 / W1_PRESCALE, bias=b1h[:, j:j + 1],
                    )

            def emit_scan(t0, t1):
                for t in range(t0, t1):
                    a = wroll[:, ((t - 1) % 4) * 512:((t - 1) % 4) * 512 + 512]
                    b_ = wroll[:, (t % 4) * 512:(t % 4) * 512 + 512]
                    nc.vector._custom_dve(lif, out=b_, in0=a,
                                          in1=h_sb[:, t * 512:(t + 1) * 512],
                                          s0=1.0, s1=0.5)
                    m = mpool.tile([128, 512], BF16, tag="m", name=f"m{t}")
                    nc.vector.tensor_scalar(m[:], b_, 1.0, None, op0=OP.is_lt)
                    masks[t] = m

            next_msum = [0]

            def flush_msum(up_to):
                while next_msum[0] < up_to:
                    emit_msum(next_msum[0])
                    next_msum[0] += 1

            for g in range(len(TCNTS)):
                if g >= 2:
                    flush_msum(OFFS[g] - 5)
                emit_group(g)
                emit_scan(OFFS[g], OFFS[g + 1])
            # scan tail: alternate spike-sum MMs with junk MMs (into a retired
            # ps_h buffer) to keep the PE HAM clock warm into mm2
            jk = ps_h.tile([128, 512], FP32, tag="ps_h", name="ps_junk")
            while next_msum[0] < T:
                emit_msum(next_msum[0])
                next_msum[0] += 1
                nc.tensor.matmul(jk[:, 0:192], ident[:], junkd[:, 0:192],
                                 start=True, stop=True)
            # warm the Exp spline table while the scan drains (scratch must
            # not alias y_sb or the y-copies' WAR defers this past mm2)
            nc.scalar.activation(ez[0:1, 1004:1008], b2_sb[0:1, 0:4], AF.Exp)
            # ssum = T - msum (spike counts are small ints, exact in fp8)
            nc.scalar.activation(ssum[:], msum[:], AF.Copy, scale=-1.0,
                                 bias=float(T))

        ssum3 = ssum[:].rearrange("p (j b) -> p j b", j=8)
        b2_32 = sb.tile([1, C], BF16)
        nc.scalar.activation(b2_32[:], b2_sb[:], AF.Copy,
                             scale=float(T) * W1_PRESCALE)
        ones = sb.tile([1, BC], BF16)
        nc.vector.memset(ones[:], 1.0)

        with tc.tile_pool(name="psy", bufs=2, space="PSUM") as psy_pool:
            for half in range(2):
                n = 512 if half == 0 else C - 512
                c0 = half * 512
                psy = psy_pool.tile([BC, 512], FP32, tag="ps_y", name=f"psy{half}")
                for pj in range(4):
                    nc.tensor.matmul(
                        psy[:, 0:n],
                        ssum3[:, 2 * pj:2 * pj + 2, :],
                        w2t3[:, 2 * pj:2 * pj + 2, c0:c0 + n],
                        start=(pj == 0), stop=False,
                        perf_mode=mybir.MatmulPerfMode.DoubleRow,
                    )
                nc.tensor.matmul(psy[:, 0:n], ones[:], b2_32[:, c0:c0 + n],
                                 start=False, stop=True)
                nc.vector.tensor_scalar(y_sb[:, c0:c0 + n], psy[:, 0:n],
                                        1.0 / (float(T) * W1_PRESCALE), None,
                                        op0=OP.mult)

        # ---- log_softmax over C (|y| small enough that no max-shift needed);
        # exp/reduce pipelined in halves across Scalar/Vector ----
        red = sb.tile([BC, 2], FP32)
        nc.scalar.activation(ez[:, 0:512], y_sb[:, 0:512], AF.Exp)
        nc.vector.reduce_sum(red[:, 0:1], ez[:, 0:512], axis=mybir.AxisListType.X)
        nc.scalar.activation(ez[:, 512:C], y_sb[:, 512:C], AF.Exp)
        nc.scalar.activation(ez[0:1, 1000:1004], b2_sb[0:1, 0:4], AF.Ln)
        nc.vector.reduce_sum(red[:, 1:2], ez[:, 512:C], axis=mybir.AxisListType.X)
        ssum_e = sb.tile([BC, 1], FP32)
        nc.vector.tensor_tensor(ssum_e[:], red[:, 0:1], red[:, 1:2], op=OP.add)
        lse = sb.tile([BC, 1], FP32)
        nc.scalar.activation(lse[:], ssum_e[:], AF.Ln)
        out_sb = sb.tile([BC, C], FP32)
        nc.vector.tensor_scalar(out_sb[:, 0:512], y_sb[:, 0:512], lse[:], None,
                                op0=OP.subtract)
        nc.sync.dma_start(y_d[:, 0:512], out_sb[:, 0:512])
        nc.vector.tensor_scalar(out_sb[:, 512:C], y_sb[:, 512:C], lse[:], None,
                                op0=OP.subtract)
        nc.sync.dma_start(y_d[:, 512:C], out_sb[:, 512:C])

    nc.compile()
    return nc


_CACHE = {}


def kernel(x, W1, b1, W2, b2):
    if "nc" not in _CACHE:
        _CACHE["nc"] = build_program()
    nc = _CACHE["nc"]

    f8 = mybir.dt.np(FP8)
    x = np.asarray(x, dtype=np.float32)
    w1t = (np.asarray(W1, dtype=np.float32).T * W1_PRESCALE).astype(f8)
    # j-major blocks: [j, p, dj, e] from W1T [dj*128+p, j*128+e]
    w1j = np.ascontiguousarray(
        w1t.reshape(8, 128, 8, 128).transpose(2, 1, 0, 3)).reshape(8, 128, 8 * 128)
    w2t = np.ascontiguousarray(
        (np.asarray(W2, dtype=np.float32).T * W1_PRESCALE).astype(f8))
    b1h = np.ascontiguousarray(
        (0.5 * np.asarray(b1, dtype=np.float32)).reshape(8, 128).T)
    b2 = np.ascontiguousarray(b2, dtype=np.float32)
    in_maps = []
    for i in range(N_CORES):
        xs = np.ascontiguousarray(
            x[:, i * BC:(i + 1) * BC, :].reshape(TB, D).T.astype(f8))
        in_maps.append({"xT": xs, "W1J": w1j, "W2T": w2t, "b1h": b1h, "b2": b2})

    res = run_bass_kernel_spmd(nc, in_maps, core_ids=list(range(N_CORES)),
                               **_CACHE.get("run_kwargs", {}))
    _CACHE["last_results"] = res
    out = np.concatenate([res.results[i]["y"] for i in range(N_CORES)], axis=0)
    return out


# revision 4
# speedup vs baseline: 1.0556x; 1.0126x over previous
"""Trainium2 Bass kernel for nn_CLFBlock (linear -> LIF scan -> linear -> T-mean -> log_softmax).

Self-contained: hardcodes shapes T=32, B=512, D=1024, C=1000 and data-parallel
sharding of the batch dim across 8 NeuronCores.

v3 schedule (ops identical to the proven baseline mix, pipeline rebuilt):
  - Host lays W1T out in j-major 128KB blocks and b1h/b2 pre-packed, so the
    DMA rings deliver exactly what the first matmul group needs first; the
    first mm1 MM issues ~1.5us after DMA flow starts instead of ~7us.
  - Junk matmuls (no data deps) run during the DMA fill to flip the PE HAM
    clock gate to 2.4GHz before real work arrives.
  - mm1 groups of tcnt=[4,4,8,8,8] timesteps: per-(g,j) PSUM tiles (1 bank,
    4-deep rotation), evicted to h_sb by Scalar ACT (scale 0.5/PRESCALE +
    0.5*b1 bias) right after each j's 4 DoubleRow MMs -> h streams into SBUF
    at 0.87us/t while the DVE scan consumes at ~1.03us/t, so the chain never
    starves and mm1 stays at the fp8-DoubleRow roofline.
  - LIF scan: fused custom DVE op (state update) + is_lt mask per step; spike
    sums accumulate on the PE (ident @ mask), deferred a few steps so mask
    waits never sit ahead of mm1 in the PE FIFO; junk MMs keep HAM warm
    through the scan tail so mm2 runs at full clock.
  - Epilogue: Exp table pre-warmed while the scan drains; y copies on the
    (idle) Vector engine; log-softmax; output DMA split in halves.
"""

import numpy as np
from contextlib import ExitStack

import concourse.bass as bass
import concourse.tile as tile
from concourse import bacc, mybir
from concourse.bass_utils import run_bass_kernel_spmd

N_CORES = 8
T, B, D, C = 32, 512, 1024, 1000
BC = B // N_CORES          # 64 rows per core
TB = T * BC                # 2048 matmul rows per core
FP32 = mybir.dt.float32
BF16 = mybir.dt.bfloat16
FP8 = mybir.dt.float8e4
W1_PRESCALE = 256.0   # host multiplies W1/W2 by this (exact power of 2) so
                      # their small uniform(-1/32,1/32) values stay in
                      # fp8e4m3's normal range; compensated on-chip
AF = mybir.ActivationFunctionType
OP = mybir.AluOpType

TCNTS = [4, 6, 6, 6, 6, 4]
OFFS = [0, 4, 10, 16, 22, 28, 32]


def _lif_op():
    """Fused LIF step: out = select(in0 < s0, in0, 0)*s1 + in1."""
    from concourse import dve_ops
    from concourse.dve_spec import Spec, Src0, Src1, Zero, C0, C1, select, lower
    from concourse.dve_uop import DveOpSpec

    for op in dve_ops.OPS:
        if op.name == "LIF_STEP_ANT":
            return op
    spec = Spec(
        body=select(Src0 < C0, Src0, Zero) * C1 + Src1,
        reference=lambda in0, in1, s0, s1, imm2: (
            np.where(in0.astype(np.float32) < s0, in0.astype(np.float32), 0.0) * s1
            + in1.astype(np.float32)).astype(np.float32),
    )
    row = dve_ops._CUSTOM_DVE_ROW_BASE + len(dve_ops.OPS)
    shas = {}
    for ver in ("v3", "v4"):
        try:
            shas[ver] = DveOpSpec(name="LIF_STEP_ANT", opcode=row,
                                  uops=lower(spec, ver=ver), rd1_en=True).sha(ver)
        except Exception:
            pass
    op = dve_ops.DveOp("LIF_STEP_ANT", spec, subdim=False, uops_sha=shas)
    dve_ops.OPS.append(op)
    dve_ops._SUB_OPCODE_FOR_NAME[op.name] = row
    dve_ops.CUSTOM_DVE_SPECS[op.name] = spec
    return op


def build_program():
    nc = bacc.Bacc("TRN2", target_bir_lowering=False, debug=False, num_devices=N_CORES)

    # W1T in j-major blocks: host layout [j=8][p=128][dj=8][e=128] so each
    # block is a contiguous 128KB transfer with 1KB rows
    w1j_d = nc.dram_tensor("W1J", [8, 128, 8 * 128], FP8, kind="ExternalInput").ap()
    xt_d = nc.dram_tensor("xT", [D, TB], FP8, kind="ExternalInput").ap()
    w2t_d = nc.dram_tensor("W2T", [D, C], FP8, kind="ExternalInput").ap()
    b1h_d = nc.dram_tensor("b1h", [128, 8], FP32, kind="ExternalInput").ap()
    b2_d = nc.dram_tensor("b2", [C], FP32, kind="ExternalInput").ap()
    y_d = nc.dram_tensor("y", [BC, C], FP32, kind="ExternalOutput").ap()

    lif = _lif_op()

    with tile.TileContext(nc) as tc, ExitStack() as ctx:
        sb = ctx.enter_context(tc.tile_pool(name="sb", bufs=1))
        mpool = ctx.enter_context(tc.tile_pool(name="mpool", bufs=10))

        # ---- constants with no DMA deps (warmup operands, scan state) ----
        io = sb.tile([128, 128], mybir.dt.int32)
        nc.gpsimd.iota(io[:], pattern=[[1, 128]], base=0, channel_multiplier=-1)
        ident = sb.tile([128, 128], BF16)
        nc.vector.tensor_scalar(ident[:], io[:], 0, None, op0=OP.is_equal)
        junkd = sb.tile([128, 512], BF16)
        nc.vector.memset(junkd[:], 0.125)
        wroll = sb.tile([128, 4 * 512], BF16)
        nc.vector.memset(wroll[:, 3 * 512:4 * 512], 0.0)   # w_{-1} = 0 in buf 3

        # ---- input DMA (two HWDGE rings, FIFO priority order):
        # b1h first (tiny, needed by the first eviction), then x cols 0-255,
        # then the 8 W1 j-blocks, then the rest of x, W2, b2 ----
        # w1t4: [p, j, dj, e]; one contiguous half-of-W1 transfer per ring
        w1t = sb.tile([128, 8 * 1024], FP8)
        w1t4 = w1t[:].rearrange("p (j k e) -> p j k e", j=8, k=8)
        w1thirds = w1t[:].rearrange("p (j x) -> p j x", j=8)
        w1src = w1j_d[:].rearrange("j p x -> p j x")
        # sync ring issues ~2us earlier than scalar: give it 6 of 8 W1 blocks
        nc.sync.dma_start(w1thirds[:, 0:6, :], w1src[:, 0:6, :])
        nc.scalar.dma_start(w1thirds[:, 6:8, :], w1src[:, 6:8, :])

        b1h = sb.tile([128, 8], FP32)
        nc.scalar.dma_start(b1h[:], b1h_d)

        xt = sb.tile([128, 8 * TB], FP8)
        xt3 = xt[:].rearrange("p (j t) -> p j t", j=8)
        xsrc = xt_d[:].rearrange("(dj p) t -> p dj t", p=128)

        def load_x_cols(c0, c1):
            nc.sync.dma_start(xt3[:, 0:4, c0:c1], xsrc[:, 0:4, c0:c1])
            nc.scalar.dma_start(xt3[:, 4:8, c0:c1], xsrc[:, 4:8, c0:c1])

        # first x chunks split across both rings right after each ring's W1
        load_x_cols(0, 256)
        load_x_cols(256, 512)
        load_x_cols(512, 1024)
        load_x_cols(1024, 1536)
        load_x_cols(1536, 2048)

        w2t = sb.tile([128, 8 * 1024], FP8)
        w2t3 = w2t[:].rearrange("p (j c) -> p j c", j=8)
        w2src = w2t_d[:].rearrange("(ej p) c -> p ej c", p=128)
        nc.sync.dma_start(w2t3[:, 0:4, 0:C], w2src[:, 0:4, :])
        nc.scalar.dma_start(w2t3[:, 4:8, 0:C], w2src[:, 4:8, :])
        b2_sb = sb.tile([1, C], FP32)
        nc.sync.dma_start(b2_sb[:], b2_d.rearrange("(a c) -> a c", a=1))

        # ---- persistent SBUF state ----
        h_sb = sb.tile([128, T * 512], BF16)
        h3 = h_sb[:].rearrange("p (t x) -> p t x", x=512)
        ssum = sb.tile([128, 512], FP8)
        y_sb = sb.tile([BC, 1024], FP32)
        ez = sb.tile([BC, 1024], FP32)

        with tc.tile_pool(name="ps_h", bufs=6, space="PSUM") as ps_h, \
             tc.tile_pool(name="psm", bufs=1, space="PSUM") as psm:

            msum = psm.tile([128, 512], FP32)
            # HAM warm-up during the DMA fill: tiny N=16 matmuls keep the PE
            # "busy" for the activity monitor at ~3% of a full MM's power (a
            # dense warmup measurably trips the chip-wide P0 power throttle
            # and slows every engine ~17%); msum's first real MM clears
            for i in range(55):
                nc.tensor.matmul(msum[:, 0:16], ident[:], junkd[:, 0:16],
                                 start=True, stop=True)

            n_msum = [0]
            masks = {}

            def emit_msum(t):
                m = masks.pop(t)
                nc.tensor.matmul(msum[:], ident[:], m[:],
                                 start=(n_msum[0] == 0), stop=(n_msum[0] == T - 1))
                n_msum[0] += 1

            def emit_group(g):
                t0, tcnt = OFFS[g], TCNTS[g]
                n = tcnt * 64
                for j in range(8):
                    ps = ps_h.tile([128, 512], FP32, tag="ps_h", name=f"ps_{g}_{j}")
                    for dp in range(4):
                        nc.tensor.matmul(
                            ps[:, 0:n],
                            w1t4[:, j, 2 * dp:2 * dp + 2, :],
                            xt3[:, 2 * dp:2 * dp + 2, t0 * 64:(t0 + tcnt) * 64],
                            start=(dp == 0), stop=(dp == 3),
                            perf_mode=mybir.MatmulPerfMode.DoubleRow,
                        )
                    nc.scalar.activation(
                        h3[:, t0:t0 + tcnt, j * 64:(j + 1) * 64],
                        ps[:, 0:n].rearrange("p (t b) -> p t b", t=tcnt),
                        AF.Identity, scale=0.5 / W1_PRESCALE, bias=b1h[:, j:j + 1],
                    )

            def emit_scan(t0, t1):
                for t in range(t0, t1):
                    a = wroll[:, ((t - 1) % 4) * 512:((t - 1) % 4) * 512 + 512]
                    b_ = wroll[:, (t % 4) * 512:(t % 4) * 512 + 512]
                    nc.vector._custom_dve(lif, out=b_, in0=a,
                                          in1=h_sb[:, t * 512:(t + 1) * 512],
                                          s0=1.0, s1=0.5)
                    m = mpool.tile([128, 512], BF16, tag="m", name=f"m{t}")
                    nc.vector.tensor_scalar(m[:], b_, 1.0, None, op0=OP.is_lt)
                    masks[t] = m

            next_msum = [0]

            def flush_msum(up_to):
                while next_msum[0] < up_to:
                    emit_msum(next_msum[0])
                    next_msum[0] += 1

            for g in range(len(TCNTS)):
                if g >= 2:
                    flush_msum(OFFS[g] - 5)
                emit_group(g)
                emit_scan(OFFS[g], OFFS[g + 1])
            # scan tail: alternate spike-sum MMs with junk MMs (into a retired
            # ps_h buffer) to keep the PE HAM clock warm into mm2
            jk = ps_h.tile([128, 512], FP32, tag="ps_h", name="ps_junk")
            while next_msum[0] < T:
                emit_msum(next_msum[0])
                next_msum[0] += 1
                nc.tensor.matmul(jk[:, 0:192], ident[:], junkd[:, 0:192],
                                 start=True, stop=True)
            # ssum = T - msum (spike counts are small ints, exact in fp8)
            nc.scalar.activation(ssum[:], msum[:], AF.Copy, scale=-1.0,
                                 bias=float(T))

        ssum3 = ssum[:].rearrange("p (j b) -> p j b", j=8)
        b2_32 = sb.tile([1, C], BF16)
        nc.scalar.activation(b2_32[:], b2_sb[:], AF.Copy,
                             scale=float(T) * W1_PRESCALE)
        ones = sb.tile([1, BC], BF16)
        nc.vector.memset(ones[:], 1.0)
        # warm the Exp spline table AFTER the Copy-family ACTs (Copy evicts
        # the spline slot); its load overlaps the mm2 matmuls
        nc.scalar.activation(ez[0:1, 1004:1008], b2_sb[0:1, 0:4], AF.Exp)

        with tc.tile_pool(name="psy", bufs=2, space="PSUM") as psy_pool:
            for half in range(2):
                n = 512 if half == 0 else C - 512
                c0 = half * 512
                psy = psy_pool.tile([BC, 512], FP32, tag="ps_y", name=f"psy{half}")
                for pj in range(4):
                    nc.tensor.matmul(
                        psy[:, 0:n],
                        ssum3[:, 2 * pj:2 * pj + 2, :],
                        w2t3[:, 2 * pj:2 * pj + 2, c0:c0 + n],
                        start=(pj == 0), stop=False,
                        perf_mode=mybir.MatmulPerfMode.DoubleRow,
                    )
                nc.tensor.matmul(psy[:, 0:n], ones[:], b2_32[:, c0:c0 + n],
                                 start=False, stop=True)
                nc.vector.tensor_scalar(y_sb[:, c0:c0 + n], psy[:, 0:n],
                                        1.0 / (float(T) * W1_PRESCALE), None,
                                        op0=OP.mult)

        # ---- log_softmax over C (|y| small enough that no max-shift needed);
        # exp/reduce pipelined in halves across Scalar/Vector ----
        red = sb.tile([BC, 2], FP32)
        nc.scalar.activation(ez[:, 0:512], y_sb[:, 0:512], AF.Exp)
        nc.vector.reduce_sum(red[:, 0:1], ez[:, 0:512], axis=mybir.AxisListType.X)
        nc.scalar.activation(ez[:, 512:C], y_sb[:, 512:C], AF.Exp)
        nc.scalar.activation(ez[0:1, 1000:1004], b2_sb[0:1, 0:4], AF.Ln)
        nc.vector.reduce_sum(red[:, 1:2], ez[:, 512:C], axis=mybir.AxisListType.X)
        ssum_e = sb.tile([BC, 1], FP32)
        nc.vector.tensor_tensor(ssum_e[:], red[:, 0:1], red[:, 1:2], op=OP.add)
        lse = sb.tile([BC, 1], FP32)
        nc.scalar.activation(lse[:], ssum_e[:], AF.Ln)
        out_sb = sb.tile([BC, C], FP32)
        nc.vector.tensor_scalar(out_sb[:, 0:512], y_sb[:, 0:512], lse[:], None,
                                op0=OP.subtract)
        nc.sync.dma_start(y_d[:, 0:512], out_sb[:, 0:512])
        nc.vector.tensor_scalar(out_sb[:, 512:C], y_sb[:, 512:C], lse[:], None,
                                op0=OP.subtract)
        nc.sync.dma_start(y_d[:, 512:C], out_sb[:, 512:C])

    nc.compile()
    return nc


_CACHE = {}


def kernel(x, W1, b1, W2, b2):
    if "nc" not in _CACHE:
        _CACHE["nc"] = build_program()
    nc = _CACHE["nc"]

    f8 = mybir.dt.np(FP8)
    x = np.asarray(x, dtype=np.float32)
    w1t = (np.asarray(W1, dtype=np.float32).T * W1_PRESCALE).astype(f8)
    # j-major blocks: [j, p, dj, e] from W1T [dj*128+p, j*128+e]
    w1j = np.ascontiguousarray(
        w1t.reshape(8, 128, 8, 128).transpose(2, 1, 0, 3)).reshape(8, 128, 8 * 128)
    w2t = np.ascontiguousarray(
        (np.asarray(W2, dtype=np.float32).T * W1_PRESCALE).astype(f8))
    b1h = np.ascontiguousarray(
        (0.5 * np.asarray(b1, dtype=np.float32)).reshape(8, 128).T)
    b2 = np.ascontiguousarray(b2, dtype=np.float32)
    in_maps = []
    for i in range(N_CORES):
        xs = np.ascontiguousarray(
            x[:, i * BC:(i + 1) * BC, :].reshape(TB, D).T.astype(f8))
        in_maps.append({"xT": xs, "W1J": w1j, "W2T": w2t, "b1h": b1h, "b2": b2})

    res = run_bass_kernel_spmd(nc, in_maps, core_ids=list(range(N_CORES)),
                               **_CACHE.get("run_kwargs", {}))
    _CACHE["last_results"] = res
    out = np.concatenate([res.results[i]["y"] for i in range(N_CORES)], axis=0)
    return out
